# revision 23
# baseline (speedup 1.0000x reference)
"""DLEM kernel for Trainium2, 8 NeuronCores, data-parallel over batch.

Pipeline per core (8 samples):
  1. Conv section on the PE (fp16 matmuls, fp32 PSUM accumulate):
     conv0(160->128) and conv1(96->96) per sample, conv2(64->64) 2-sample
     block-diag, conv3(32->32) + all transposed convs 4-sample block-diag,
     mixer folded into 4 sparse [128,36] matmuls. PSUM->SBUF evacuation
     with fused bias+relu alternating ACT/DVE. Inputs are cast to fp16 on
     the host; next-sample input DMAs are prefetched one iteration ahead
     across the sync/scalar/gpsimd queues.
  2. DLEM scan as a linear 2-term recurrence without division:
         u_{d+1}[j] = Q(d,j)*u_d[j] + P(d,j+1)*u_d[j+1],
         P(d,j) = 1 / (1 + left[d+j]/right[j]),  Q = 1 - P
     Layout: partition p = 16*sample + chunk, free dim = 128 positions +
     16-col halo; halo refreshed every 15 steps by a DVE stream_shuffle.
     State in fp16 (DVE 2x mode): 2 DVE tensor_tensor ops per step:
     one fused double-width mult computing [a|b] = [P|Q] (*) [u|u]
     (P,Q concatenated per table row; u read twice via a stride-0 mid
     AP dim), then u' = b + shift(a) reading both halves of ab.
     P/Q built per 64-step block, double-buffered:
       den = ln r - ln l (skewed AP) as fp16 2x-mode ops ON THE DVE
       (Pool/GpSimd compute stalls the DVE ~1:1, so it is never used
       during the scan), P/Q = sigmoid(-/+den) on ACT (ACT runs fully
       parallel to DVE). Block 0 uses fine-grained first bands so step 1
       starts sooner after the conv.
  3. Fixup y = const^510 * u / (right + left[511+j]); outputs DMA'd over
     three queues.
"""
import os
import sys

for _p in ("/opt/trn_rl_repo", "/root/.axon_site/_ro/trn_rl_repo"):
    if os.path.isdir(_p) and _p not in sys.path:
        sys.path.insert(0, _p)

import numpy as np

B = 64
NCORES = 8
S = B // NCORES          # samples per core
N = 2048
EPI, SEQD, CPR = 128, 32, 32
NSTEP = 510              # d = 1..510
H = 16                   # halo columns
FW = 128 + H             # scan tile free width
C = 16                   # chunks per sample
BLK = 64                 # P-table block depth
NBLK = (NSTEP + BLK - 1) // BLK      # 8 blocks (last = 62 rows)
RFR = 15                 # halo refresh period (H - 1)
LEXT = 2700              # left_ext width
GEXT = C * 128 + 64      # 2112, right/curr width
LRW = 656                # l_row width  (needs 449+61+143 = 653)
NBAND = 1                # bulk build bands per block

_LBR = (2046, 2044, 2042, 2040)   # branch initial lengths
_T0S = (3, 4, 5, 6)               # branch initial data col offsets


# ---------------------------------------------------------------- weights --
def _pack_weights(inp):
    """All numpy-side weight packing. lhsT tensors are (K, 3, M)."""
    d = {}
    w0 = inp['conv0_w']            # (128, 160, 3)
    perm0 = np.concatenate([np.arange(32, 128), np.arange(0, 32)])
    d['c0A'] = np.ascontiguousarray(np.transpose(w0[perm0][:, :128, :], (1, 2, 0)))
    # c0B: K = (k, seq-ch) stacked 96 rows for the shift-stacked seq tile
    c0b_stk = np.zeros((96, 128), np.float32)
    for k in range(3):
        c0b_stk[32 * k:32 * k + 32, :] = w0[perm0][:, 128:, k].T
    d['c0B'] = c0b_stk
    d['c0bl'] = inp['conv0_b'][0:32].reshape(32, 1)
    d['c0bh'] = inp['conv0_b'][32:128].reshape(96, 1)

    w1 = inp['conv1_w']            # (96, 96, 3)
    perm1 = np.concatenate([np.arange(32, 96), np.arange(0, 32)])
    d['c1'] = np.ascontiguousarray(np.transpose(w1[perm1], (1, 2, 0)))
    d['c1bl'] = inp['conv1_b'][0:32].reshape(32, 1)
    d['c1bh'] = inp['conv1_b'][32:96].reshape(64, 1)

    w2 = inp['conv2_w']            # (64, 64, 3)
    l2 = np.zeros((128, 3, 128), np.float32)
    for k in range(3):
        t = w2[:, :, k].T          # (ci, co)
        l2[0:64, k, 0:32] = t[:, 0:32]
        l2[0:64, k, 64:96] = t[:, 32:64]
        l2[64:128, k, 32:64] = t[:, 0:32]
        l2[64:128, k, 96:128] = t[:, 32:64]
    d['c2'] = l2
    b2 = np.zeros((128, 1), np.float32)
    b2[0:32, 0] = inp['conv2_b'][0:32]
    b2[32:64, 0] = inp['conv2_b'][0:32]
    b2[64:96, 0] = inp['conv2_b'][32:64]
    b2[96:128, 0] = inp['conv2_b'][32:64]
    d['c2bl'] = b2[0:64]
    d['c2bh'] = b2[64:128]

    w3 = inp['conv3_w']            # (32, 32, 3)
    l3 = np.zeros((128, 3, 128), np.float32)
    for k in range(3):
        t = w3[:, :, k].T
        for sl in range(4):
            l3[32 * sl:32 * sl + 32, k, 32 * sl:32 * sl + 32] = t
    d['c3'] = l3
    d['c3b'] = np.tile(inp['conv3_b'], 4).reshape(128, 1)

    for i in range(4):
        w = inp[f'tconv{i}_w']     # (ci=32, co=32, 3) torch layout
        wf = np.flip(w, -1).transpose(1, 0, 2)   # (co, ci, 3)
        lt = np.zeros((128, 3, 128), np.float32)
        for k in range(3):
            t = wf[:, :, k].T      # (ci, co)
            for sl in range(4):
                lt[32 * sl:32 * sl + 32, k, 32 * sl:32 * sl + 32] = t
        d[f't{i}'] = lt
        d[f't{i}b'] = np.tile(inp[f'tconv{i}_b'], 4).reshape(128, 1)

    mw = inp['mixer_w'][:, :, 0]   # (2, 128)
    for i in range(4):
        m = np.zeros((128, 36), np.float32)
        for sl in range(4):
            m[32 * sl:32 * sl + 32, sl] = mw[0, 32 * i:32 * i + 32]      # left
            m[32 * sl:32 * sl + 32, 32 + sl] = mw[1, 32 * i:32 * i + 32]  # right
        d[f'mx{i}'] = m
    d['mxbl'] = np.full((4, 1), inp['mixer_b'][0], np.float32)
    d['mxbr'] = np.full((4, 1), inp['mixer_b'][1], np.float32)
    for k in list(d):
        if not (k.endswith('b') or 'bl' in k or 'bh' in k
                or k in ('mxbl', 'mxbr')):
            d[k] = d[k].astype(np.float16)
    return d


_WSHAPES = {'c0A': [128, 3, 128],
            'c0bl': [32, 1], 'c0bh': [96, 1], 'c0B': [96, 128],
            'c1': [96, 3, 96], 'c1bl': [32, 1], 'c1bh': [64, 1],
            'c2': [128, 3, 128], 'c2bl': [64, 1], 'c2bh': [64, 1],
            'c3': [128, 3, 128], 'c3b': [128, 1],
            'mxbl': [4, 1], 'mxbr': [4, 1]}
for _i in range(4):
    _WSHAPES[f't{_i}'] = [128, 3, 128]
    _WSHAPES[f't{_i}b'] = [128, 1]
    _WSHAPES[f'mx{_i}'] = [128, 36]


def coltiles(L, maxw=512):
    out, off = [], 0
    while off < L:
        w = min(maxw, L - off)
        out.append((off, w))
        off += w
    return out


# ---------------------------------------------------------------- program --
def build_program(const_val):
    import bass_rust
    import concourse.bacc as bacc
    import concourse.mybir as mybir
    from concourse.tile import TileContext

    f32 = mybir.dt.float32
    f32r = mybir.dt.float32r
    f16 = mybir.dt.float16
    AF = mybir.ActivationFunctionType
    OP = mybir.AluOpType

    def cap(ap, dims, offset=None):
        b = ap.copy()
        b.ap = bass_rust.VecI64Pair(dims)
        if offset is not None:
            b.offset = offset
        return b

    nc = bacc.Bacc("TRN2", target_bir_lowering=False, debug=False,
                   num_devices=NCORES)

    sig = nc.declare_dram_parameter("signal", [S, EPI, N], f16, isOutput=False)
    seq = nc.declare_dram_parameter("seq", [S, SEQD, N], f16, isOutput=False)
    curr = nc.declare_dram_parameter("curr", [S, N - 1], f32, isOutput=False)
    wd = {k: nc.declare_dram_parameter(
              k, sh, f32 if (k.endswith('b') or 'bl' in k or 'bh' in k
                             or k in ('mxbl', 'mxbr')) else f16,
              isOutput=False)
          for k, sh in _WSHAPES.items()}
    yout = nc.declare_dram_parameter("y", [S, 1537], f32, isOutput=True)

    L0, L1, L2, L3 = _LBR[0], _LBR[1], _LBR[2], _LBR[3]
    BW = 2052

    with TileContext(nc) as tc:
        with (tc.tile_pool(name="wp", bufs=1) as wp,
              tc.tile_pool(name="lrp", bufs=1) as lrp,
              tc.tile_pool(name="psp", bufs=4, space="PSUM") as psp,
              tc.tile_pool(name="psq", bufs=2, space="PSUM") as psq):

            wt = {}
            for k, sh in _WSHAPES.items():
                t = wp.tile(sh,
                            f32 if (k.endswith('b') or 'bl' in k or 'bh' in k
                                    or k in ('mxbl', 'mxbr')) else f16,
                            name=f"w_{k}", tag=f"w_{k}")
                eng = nc.scalar if (k.startswith('c0') or k.startswith('c1')) \
                    else nc.gpsimd
                eng.dma_start(out=t[:], in_=wd[k][:])
                wt[k] = t

            # long-lived scan-side tiles (coexist with conv tiles)
            left_ext = lrp.tile([36, LEXT], f16, tag="left_ext")    # g0 rows 0:4, g1 rows 32:36
            right_ext = lrp.tile([36, GEXT], f16, tag="right_ext")
            curr1 = lrp.tile([36, GEXT], f32, tag="curr1")
            r_row = lrp.tile([128, FW], f16, tag="r_row")
            lnwarm = lrp.tile([1, 1], f32, tag="lnwarm")
            curr_row = lrp.tile([128, FW], f32, tag="curr_row")
            l_row = lrp.tile([128, LRW], f16, tag="l_row")
            # init only the tail columns (mixer/DMA fill [0, N)); unused
            # rows are never read by the skew gathers.
            nc.gpsimd.memset(left_ext[:, N:LEXT], 1.0)
            nc.gpsimd.memset(right_ext[:, N:GEXT], 0.0)
            nc.gpsimd.memset(curr1[:, N - 1:GEXT], 0.0)
            for g in range(2):
                nc.gpsimd.dma_start(out=curr1[32 * g:32 * g + 4, 0:N - 1],
                                    in_=curr[4 * g:4 * g + 4])

            # ------------------------ conv section ------------------------
            with (tc.tile_pool(name="actp", bufs=1) as actp,
                  tc.tile_pool(name="iop", bufs=2) as iop):
                def load_inp(s):
                    # split the big signal load across two DMA queues; for
                    # sample 0, sequence the sync queue so the first conv0
                    # PSUM group (x0a cols 0:514 + seq cols 0:512) unblocks
                    # earliest; gpsimd is busy with weights then.
                    x0a = iop.tile([128, N], f16, name="x0a", tag="x0a")
                    seq_stk = iop.tile([96, L0], f16, name="seq_stk",
                                       tag="seq_stk")
                    if s == 0:
                        nc.sync.dma_start(out=x0a[:, 0:640],
                                          in_=sig[s][:, 0:640])
                        for k in range(3):
                            nc.sync.dma_start(
                                out=seq_stk[32 * k:32 * k + 32, :],
                                in_=seq[s][:, k:k + L0])
                        nc.sync.dma_start(out=x0a[:, 640:N // 2],
                                          in_=sig[s][:, 640:N // 2])
                        nc.scalar.dma_start(out=x0a[:, N // 2:N],
                                            in_=sig[s][:, N // 2:N])
                    else:
                        nc.sync.dma_start(out=x0a[:, 0:N // 2],
                                          in_=sig[s][:, 0:N // 2])
                        nc.gpsimd.dma_start(out=x0a[:, N // 2:N],
                                            in_=sig[s][:, N // 2:N])
                        for k in range(3):
                            eng = nc.sync if k != 1 else nc.gpsimd
                            eng.dma_start(out=seq_stk[32 * k:32 * k + 32, :],
                                          in_=seq[s][:, k:k + L0])
                    return x0a, seq_stk

                for g in range(2):
                    b_init = [actp.tile([128, BW], f16, name=f"binit{i}", tag=f"binit{i}")
                              for i in range(4)]
                    b_alt = [actp.tile([128, BW], f16, name=f"balt{i}", tag=f"balt{i}")
                             for i in range(4)]
                    zsrc = wt['c0A'][:, 0, 0:8]
                    for i in range(4):
                        for t in (b_init[i], b_alt[i]):
                            nc.scalar.activation(t[:, 0:8], zsrc, AF.Copy,
                                                 bias=0.0, scale=0.0)
                            nc.scalar.activation(t[:, BW - 8:BW], zsrc, AF.Copy,
                                                 bias=0.0, scale=0.0)

                    conv2in = [actp.tile([128, L1], f16, name=f"c2in{p}", tag=f"c2in{p}")
                               for p in range(2)]
                    conv3in = actp.tile([128, L2], f16, tag="c3in")

                    def evac(ci, dst, src, bias, act_first=True):
                        """relu(src + bias) -> dst, alternating ACT/DVE."""
                        if (ci % 2 == 0) == act_first:
                            nc.scalar.activation(dst, src, AF.Relu,
                                                 bias=bias, scale=1.0)
                        else:
                            nc.vector.tensor_scalar(dst, src, bias, 0.0,
                                                    OP.add, OP.max)

                    for sl in range(4):
                        s = 4 * g + sl
                        if s == 0:
                            cur_inp = load_inp(0)
                        x0a, seq_stk = cur_inp
                        # prefetch next sample (crosses the g boundary too)
                        cur_inp = load_inp(s + 1) if s + 1 < S else None
                        rest1 = iop.tile([96, L0], f16, tag="rest1")

                        # conv0: 160->128 (3 sig shifts + 1 stacked-seq matmul)
                        for ci, (off, w) in enumerate(coltiles(L0)):
                            ps = psp.tile([128, 512], f32, tag="ps")
                            for k in range(3):
                                nc.tensor.matmul(
                                    out=ps[:, 0:w], lhsT=(wt['c0A'][:, k, :]),
                                    rhs=(x0a[:, off + k:off + k + w]),
                                    start=(k == 0), stop=False)
                            nc.tensor.matmul(
                                out=ps[:, 0:w], lhsT=(wt['c0B'][:]),
                                rhs=(seq_stk[:, off:off + w]),
                                start=False, stop=True)
                            evac(ci, b_init[0][32 * sl:32 * sl + 32,
                                               3 + off:3 + off + w],
                                 ps[96:128, 0:w], wt['c0bl'][:])
                            evac(ci + 1, rest1[:, off:off + w],
                                 ps[0:96, 0:w], wt['c0bh'][:])

                        # conv1: 96->96
                        for ci, (off, w) in enumerate(coltiles(L1)):
                            ps1 = psq.tile([96, 512], f32, tag="ps1")
                            for k in range(3):
                                nc.tensor.matmul(
                                    out=ps1[:, 0:w], lhsT=(wt['c1'][:, k, :]),
                                    rhs=(rest1[:, off + k:off + k + w]),
                                    start=(k == 0), stop=(k == 2))
                            evac(ci, b_init[1][32 * sl:32 * sl + 32,
                                               4 + off:4 + off + w],
                                 ps1[64:96, 0:w], wt['c1bl'][:])
                            evac(ci + 1,
                                 conv2in[sl // 2][64 * (sl % 2):64 * (sl % 2) + 64,
                                                  off:off + w],
                                 ps1[0:64, 0:w], wt['c1bh'][:])

                    # conv2 per pair (2 samples block-diag)
                    for p in range(2):
                        for ci, (off, w) in enumerate(coltiles(L2)):
                            ps = psp.tile([128, 512], f32, tag="ps")
                            for k in range(3):
                                nc.tensor.matmul(
                                    out=ps[:, 0:w], lhsT=(wt['c2'][:, k, :]),
                                    rhs=(conv2in[p][:, off + k:off + k + w]),
                                    start=(k == 0), stop=(k == 2))
                            evac(ci, b_init[2][64 * p:64 * p + 64,
                                               5 + off:5 + off + w],
                                 ps[0:64, 0:w], wt['c2bl'][:])
                            evac(ci + 1, conv3in[64 * p:64 * p + 64, off:off + w],
                                 ps[64:128, 0:w], wt['c2bh'][:])

                    # conv3 (4-sample block-diag)
                    for ci, (off, w) in enumerate(coltiles(L3)):
                        ps = psp.tile([128, 512], f32, tag="ps")
                        for k in range(3):
                            nc.tensor.matmul(
                                out=ps[:, 0:w], lhsT=(wt['c3'][:, k, :]),
                                rhs=(conv3in[:, off + k:off + k + w]),
                                start=(k == 0), stop=(k == 2))
                        evac(ci, b_init[3][:, 6 + off:6 + off + w],
                             ps[:, 0:w], wt['c3b'][:])

                    # transposed conv chains (4-sample block-diag)
                    finals = []
                    for i in range(4):
                        t0, L = _T0S[i], _LBR[i]
                        src, dst = b_init[i], b_alt[i]
                        for ti in range(3 - i, 4):
                            Lo = L + 2
                            for ci, (off, w) in enumerate(coltiles(Lo)):
                                ps = psp.tile([128, 512], f32, tag="ps")
                                for k in range(3):
                                    nc.tensor.matmul(
                                        out=ps[:, 0:w],
                                        lhsT=(wt[f't{ti}'][:, k, :]),
                                        rhs=(src[:, t0 - 2 + off + k:
                                                 t0 - 2 + off + k + w]),
                                        start=(k == 0), stop=(k == 2))
                                evac(ci, dst[:, t0 - 1 + off:t0 - 1 + off + w],
                                     ps[:, 0:w], wt[f't{ti}b'][:])
                            src, dst = dst, src
                            t0, L = t0 - 1, L + 2
                        finals.append(src)   # final 2048 cols at [2, 2050)

                    # mixer + sigmoid -> left/right rows directly
                    for (off, w) in coltiles(N):
                        psm = psq.tile([36, 512], f32, tag="psm")
                        for i in range(4):
                            nc.tensor.matmul(
                                out=psm[:, 0:w], lhsT=(wt[f'mx{i}'][:]),
                                rhs=(finals[i][:, 2 + off:2 + off + w]),
                                start=(i == 0), stop=(i == 3))
                        nc.scalar.activation(left_ext[32 * g:32 * g + 4, off:off + w],
                                             psm[0:4, 0:w], AF.Sigmoid,
                                             bias=wt['mxbl'][:], scale=1.0)
                        nc.scalar.activation(right_ext[32 * g:32 * g + 4, off:off + w],
                                             psm[32:36, 0:w], AF.Sigmoid,
                                             bias=wt['mxbr'][:], scale=1.0)

                    if g == 1:
                        # preload the Ln act table while skew DMAs run; the
                        # real Ln ops then skip their 1.3us table load
                        nc.scalar.activation(lnwarm[:], wt['c0bl'][0:1],
                                             AF.Ln, bias=1.0, scale=1.0)
                    # skew DMAs into scan layout (partition p = 16s + c);
                    # one DMA per tensor per group (4 source rows each)
                    sqengs = [nc.gpsimd, nc.sync, nc.scalar]
                    for qi, (src_t, dst_t, width) in enumerate(
                            ((curr1, curr_row, FW), (right_ext, r_row, FW),
                             (left_ext, l_row, LRW))):
                        src_pitch = src_t.shape[1]
                        in_ap = cap(src_t[:],
                                    [[src_pitch, 4], [128, C], [1, width]],
                                    offset=32 * g * src_pitch)
                        sqengs[qi % 3].dma_start(
                            out=dst_t[64 * g:64 * g + 64, 0:width],
                            in_=in_ap)

            # ------------------------ scan section ------------------------
            with tc.tile_pool(name="scanp", bufs=1) as scanp:
                u_t = scanp.tile([128, FW], f16, tag="u_t")
                ab_t = scanp.tile([128, 2 * FW], f16, tag="ab_t")
                ll_row = scanp.tile([128, LRW], f16, tag="ll_row")
                lr_row = scanp.tile([128, FW], f16, tag="lr_row")
                den = scanp.tile([128, BLK, FW], f16, tag="den")
                # P and Q concatenated per row: [P(0:FW) | Q(0:FW)]
                pqbuf = [scanp.tile([128, BLK, 2 * FW], f16, name=f"pqbuf{i}",
                                    tag=f"pqbuf{i}")
                         for i in range(2)]

                # log-space left/right (guarded against exact zeros)
                epsb = scanp.tile([128, 1], f32, tag="epsb")
                nc.gpsimd.memset(epsb[:], 1e-30)
                nc.scalar.activation(ll_row[:], l_row[:], AF.Ln, bias=epsb[:],
                                     scale=1.0)
                nc.scalar.activation(lr_row[:], r_row[:], AF.Ln, bias=epsb[:],
                                     scale=1.0)
                # u init: u = curr * (right + left[pos+1])  (fp16)
                nc.vector.tensor_tensor(out=den[:, 0, 0:FW], in0=r_row[:],
                                        in1=l_row[:, 1:FW + 1], op=OP.add)
                nc.vector.tensor_tensor(out=u_t[:], in0=den[:, 0, 0:FW],
                                        in1=curr_row[:], op=OP.mult)

                def build_block(blk):
                    # P(d,j) = sigmoid(ln r[j] - ln l[d+j]), Q = 1 - P
                    # subtract on DVE (Pool compute stalls DVE ~1:1), sigmoids
                    # on ACT (runs fully parallel to DVE).
                    d0 = 1 + BLK * blk
                    nrows = min(BLK, NSTEP - BLK * blk)
                    pq = pqbuf[blk % 2]
                    if blk == 0:
                        # fine-grained first bands so step 1 starts sooner
                        bounds = [0, 8, 16, 32, 48, nrows]
                    else:
                        bounds = [nrows * i // NBAND for i in range(NBAND + 1)]
                    for i in range(len(bounds) - 1):
                        r0, r1 = bounds[i], bounds[i + 1]
                        nsub = r1 - r0
                        lsk = cap(ll_row[:], [[LRW, 128], [1, nsub], [1, FW]],
                                  offset=d0 + r0)
                        rbc = cap(lr_row[:], [[FW, 128], [0, nsub], [1, FW]],
                                  offset=0)
                        d3 = den[:, r0:r1, :]
                        nc.vector.tensor_tensor(out=d3, in0=lsk, in1=rbc,
                                                op=OP.subtract)
                        pdst = cap(pq[:], [[BLK * 2 * FW, 128], [2 * FW, nsub],
                                           [1, FW]], offset=r0 * 2 * FW)
                        qdst = cap(pq[:], [[BLK * 2 * FW, 128], [2 * FW, nsub],
                                           [1, FW]], offset=r0 * 2 * FW + FW)
                        nc.scalar.activation(pdst, d3, AF.Sigmoid,
                                             bias=0.0, scale=-1.0)
                        nc.scalar.activation(qdst, d3, AF.Sigmoid,
                                             bias=0.0, scale=1.0)

                build_block(0)
                stepno = 0
                V = FW
                for blk in range(NBLK):
                    nrows = min(BLK, NSTEP - BLK * blk)
                    if blk + 1 < NBLK:
                        build_block(blk + 1)
                    pqc = pqbuf[blk % 2]
                    for r in range(nrows):
                        # fused a|b: one 2V-wide op; u read twice via a
                        # stride-0 mid dim. a -> ab[0:V], b -> ab[FW:FW+V].
                        abw = cap(ab_t[:], [[2 * FW, 128], [FW, 2], [1, V]])
                        pqr = cap(pqc[:], [[BLK * 2 * FW, 128], [FW, 2],
                                           [1, V]], offset=r * 2 * FW)
                        u2 = cap(u_t[:], [[FW, 128], [0, 2], [1, V]])
                        nc.vector.tensor_tensor(out=abw, in0=pqr, in1=u2,
                                                op=OP.mult)
                        nc.vector.tensor_tensor(out=u_t[:, 0:V - 1],
                                                in0=ab_t[:, FW:FW + V - 1],
                                                in1=ab_t[:, 1:V], op=OP.add)
                        V -= 1
                        stepno += 1
                        if stepno % RFR == 0 and stepno < NSTEP:
                            nc.vector.stream_shuffle(
                                out=u_t[:, 128:FW], in_=u_t[:, 0:H],
                                mask=list(range(1, 32)) + [31])
                            V = FW

                # fixup: y = const^NSTEP * u / (right + left[511+j])
                fin = scanp.tile([128, 128], f32, tag="fin")
                fin2 = scanp.tile([128, 128], f32, tag="fin2")
                nc.vector.tensor_tensor(out=fin[:], in0=r_row[:, 0:128],
                                        in1=l_row[:, 511:511 + 128], op=OP.add)
                nc.vector.reciprocal_approx_fast(out=fin2[:], in_=fin[:])
                nc.vector.tensor_copy(fin[:], u_t[:, 0:128])
                nc.vector.tensor_tensor(out=fin[:], in0=fin[:],
                                        in1=fin2[:], op=OP.mult)
                cpow = float(const_val) ** NSTEP
                nc.vector.tensor_scalar_mul(fin2[:], fin[:], cpow)
                qengs = [nc.sync, nc.scalar, nc.gpsimd]
                for s_all in range(S):
                    qengs[s_all % 3].dma_start(
                        out=yout[s_all:s_all + 1, 0:1536],
                        in_=fin2[16 * s_all:16 * s_all + 12, :])
                    qengs[(s_all + 1) % 3].dma_start(
                        out=yout[s_all:s_all + 1, 1536:1537],
                        in_=fin2[16 * s_all + 12:16 * s_all + 13, 0:1])

    nc.compile()
    return nc


_CACHE = {}


def _get_program(const_val):
    key = round(float(const_val), 8)
    if key not in _CACHE:
        _CACHE[key] = build_program(const_val)
    return _CACHE[key]


def make_in_maps(inputs):
    inputs = {k: np.asarray(v) for k, v in inputs.items()}
    wpack = _pack_weights(inputs)
    in_maps = []
    for core in range(NCORES):
        sl = slice(core * S, core * S + S)
        m = {'signal': np.ascontiguousarray(
                 inputs['signal'][sl]).astype(np.float16),
             'seq': np.ascontiguousarray(
                 inputs['seq'][sl]).astype(np.float16),
             'curr': np.ascontiguousarray(inputs['curr_diag'][sl])}
        m.update(wpack)
        in_maps.append(m)
    return in_maps


def kernel(**inputs):
    const_val = float(np.asarray(inputs['const']))
    nc = _get_program(const_val)
    in_maps = make_in_maps(inputs)
    from concourse.bass_utils import run_bass_kernel_spmd
    res = run_bass_kernel_spmd(nc, in_maps, list(range(NCORES)))
    out = np.concatenate([res.results[i]['y'] for i in range(NCORES)], axis=0)
    return out.astype(np.float32)

